# revision 1
# baseline (speedup 1.0000x reference)
"""Adaptive embedding (nn_AdaptiveEmbedding) Trainium2 Bass kernel.

Strategy: token-shard across 8 NeuronCores. Host routes each token to one of
6 vocab groups (cluster 1 and 2 are split in half so local row indices fit
int16 for dma_gather), pads each group's token list so every core gets an
identical count, and stages:
  - bf16 tables (narrow tables padded to 128 cols so gather rows are >=256B)
  - bf16 pre-transposed projections (scaled by sqrt(D_PROJ)), chunk-major
  - per-core int16 gather index tensors (16-partition wrapped, replicated)
Device (per core): dma_gather(transpose=True) pulls embedding rows already
transposed [d, tokens]; TensorE contracts d against the projection chunks
into PSUM [tokens, 1024]; DVE copies to SBUF; DMA writes output rows.
Host scatters the per-core rows back to original token positions.
"""

import math

import numpy as np
import ml_dtypes

import concourse.bacc as bacc
import concourse.mybir as mybir
import concourse.tile as tile
from concourse.bass_utils import run_bass_kernel_spmd

N_CORES = 8
D_PROJ = 1024
EMB_SCALE = float(D_PROJ) ** 0.5

# (token_left, token_right, table_name, row_lo, row_hi, d, proj_index)
GROUPS = [
    (0, 20000, "emb0", 0, 20000, 1024, 0),
    (20000, 40000, "emb1", 0, 20000, 256, 1),
    (40000, 60000, "emb1", 20000, 40000, 256, 1),
    (60000, 80000, "emb2", 0, 20000, 64, 2),
    (80000, 100000, "emb2", 20000, 40000, 64, 2),
    (100000, 128000, "emb3", 0, 28000, 16, 3),
]

BF16 = ml_dtypes.bfloat16

# Module-level handle for test harness inspection (exec_time_ns etc).
LAST_RESULT = None


def _elem_size(d):
    """Gathered row length in bf16 elements (>=256B granularity)."""
    return max(d, 128)


def _build(ns, caps, nblocks):
    """Build the SPMD Bass graph. ns[g]=tokens per core per group,
    caps[g]=gather count (ns rounded up to 128), nblocks[g]=ceil(ns/128)."""
    nc = bacc.Bacc("TRN2", target_bir_lowering=False, num_devices=N_CORES)

    embs, idxs = [], []
    for g, (_, _, tname, rlo, rhi, d, pj) in enumerate(GROUPS):
        if ns[g] == 0:
            embs.append(None)
            idxs.append(None)
            continue
        v = rhi - rlo
        embs.append(
            nc.dram_tensor(f"t{g}", [v, _elem_size(d)], mybir.dt.bfloat16,
                           kind="ExternalInput")
        )
        idxs.append(
            nc.dram_tensor(f"i{g}", [128, caps[g] // 16], mybir.dt.int16,
                           kind="ExternalInput")
        )

    pjts = []
    used_pj = {GROUPS[g][6] for g in range(len(GROUPS)) if ns[g] > 0}
    pj_dims = [1024, 256, 64, 16]
    for pj, d in enumerate(pj_dims):
        if pj not in used_pj:
            pjts.append(None)
            continue
        part = min(d, 128)
        nchunk = max(d // 128, 1)
        pjts.append(
            nc.dram_tensor(f"p{pj}", [part, nchunk * D_PROJ], mybir.dt.bfloat16,
                           kind="ExternalInput")
        )

    total_rows = sum(ns)
    out = nc.dram_tensor("out", [total_rows, D_PROJ], mybir.dt.float32,
                         kind="ExternalOutput")

    with tile.TileContext(nc) as tc:
        with (
            tc.tile_pool(name="const", bufs=1) as cpool,
            tc.tile_pool(name="work", bufs=3) as wpool,
            tc.tile_pool(name="psum", bufs=4, space="PSUM") as ppool,
        ):
            pjt_t = [None] * 4
            for pj, d in enumerate(pj_dims):
                if pjts[pj] is None:
                    continue
                part = min(d, 128)
                nchunk = max(d // 128, 1)
                t = cpool.tile([part, nchunk * D_PROJ], mybir.dt.bfloat16,
                               tag=f"pjt{pj}")
                nc.sync.dma_start(t[:], pjts[pj][:])
                pjt_t[pj] = t

            idx_t, et_t = [None] * len(GROUPS), [None] * len(GROUPS)
            for g, (_, _, _, rlo, rhi, d, pj) in enumerate(GROUPS):
                if ns[g] == 0:
                    continue
                it = cpool.tile([128, caps[g] // 16], mybir.dt.int16, tag=f"idx{g}")
                nc.sync.dma_start(it[:], idxs[g][:])
                idx_t[g] = it
                es = _elem_size(d)
                nchunk = es // 128
                et = cpool.tile([128, nchunk, caps[g]], mybir.dt.bfloat16,
                                tag=f"et{g}")
                nc.gpsimd.dma_gather(et[:], embs[g][:], it[:], caps[g], caps[g],
                                     es, transpose=True)
                et_t[g] = et

            off = 0
            for g, (_, _, _, rlo, rhi, d, pj) in enumerate(GROUPS):
                if ns[g] == 0:
                    continue
                kdim = min(d, 128)
                nchunk = max(d // 128, 1)
                for b in range(nblocks[g]):
                    m = min(128, ns[g] - b * 128)
                    ot = wpool.tile([128, D_PROJ], mybir.dt.float32, tag="ot")
                    for h in range(2):
                        ps = ppool.tile([128, 512], mybir.dt.float32, tag="ps")
                        for c in range(nchunk):
                            nc.tensor.matmul(
                                ps[:m, :],
                                et_t[g][:kdim, c, b * 128: b * 128 + m],
                                pjt_t[pj][:kdim, c * D_PROJ + h * 512:
                                          c * D_PROJ + h * 512 + 512],
                                start=(c == 0),
                                stop=(c == nchunk - 1),
                            )
                        nc.vector.tensor_copy(ot[:m, h * 512:(h + 1) * 512],
                                              ps[:m, :])
                    nc.sync.dma_start(
                        out[off + b * 128: off + b * 128 + m, :], ot[:m, :])
                off += ns[g]

    nc.compile()
    return nc


def kernel(input, emb0, emb1, emb2, emb3, proj0, proj1, proj2, proj3):
    global LAST_RESULT
    inp = np.asarray(input)
    flat = inp.reshape(-1).astype(np.int64)
    T = flat.shape[0]
    tables = {"emb0": np.asarray(emb0), "emb1": np.asarray(emb1),
              "emb2": np.asarray(emb2), "emb3": np.asarray(emb3)}
    projs = [np.asarray(proj0), np.asarray(proj1),
             np.asarray(proj2), np.asarray(proj3)]

    # --- route tokens to groups -------------------------------------------
    ns, caps, nblocks = [], [], []
    pos_cores, loc_cores = [], []  # per group: [8, n_g] arrays
    for (l, r, tname, rlo, rhi, d, pj) in GROUPS:
        sel = (flat >= l) & (flat < r)
        pos = np.nonzero(sel)[0]
        loc = (flat[pos] - l).astype(np.int64)
        n = int(math.ceil(len(pos) / N_CORES)) if len(pos) else 0
        ns.append(n)
        caps.append(((n + 127) // 128) * 128 if n else 0)
        nblocks.append((n + 127) // 128 if n else 0)
        pad = n * N_CORES - len(pos)
        pos_p = np.concatenate([pos, np.full(pad, -1, np.int64)])
        loc_p = np.concatenate([loc, np.zeros(pad, np.int64)])
        pos_cores.append(pos_p.reshape(N_CORES, n) if n else None)
        loc_cores.append(loc_p.reshape(N_CORES, n) if n else None)

    # --- stage tables (bf16, narrow rows padded to 128 cols) ---------------
    emb_stage = []
    for g, (l, r, tname, rlo, rhi, d, pj) in enumerate(GROUPS):
        if ns[g] == 0:
            emb_stage.append(None)
            continue
        sl = tables[tname][rlo:rhi].astype(BF16)
        es = _elem_size(d)
        if es != d:
            padded = np.zeros((sl.shape[0], es), dtype=BF16)
            padded[:, :d] = sl
            sl = padded
        emb_stage.append(np.ascontiguousarray(sl))

    # --- stage projections: (proj.T * EMB_SCALE), chunk-major bf16 ---------
    pjt_stage = [None] * 4
    for pj in range(4):
        d = projs[pj].shape[1]
        pt = (projs[pj].T.astype(np.float32) * EMB_SCALE)  # [d, D_PROJ]
        if d >= 128:
            nchunk = d // 128
            pt = pt.reshape(nchunk, 128, D_PROJ).transpose(1, 0, 2)
            pt = pt.reshape(128, nchunk * D_PROJ)
        pjt_stage[pj] = np.ascontiguousarray(pt.astype(BF16))

    # --- per-core index tensors -------------------------------------------
    idx_stage = []  # [group][core] -> [128, cap/16] int16
    for g in range(len(GROUPS)):
        if ns[g] == 0:
            idx_stage.append(None)
            continue
        cap = caps[g]
        per_core = []
        for k in range(N_CORES):
            loc = loc_cores[g][k]
            full = np.zeros(cap, np.int16)
            full[:ns[g]] = loc.astype(np.int16)
            wrapped = np.tile(full.reshape(cap // 16, 16).T, (8, 1))
            per_core.append(np.ascontiguousarray(wrapped))
        idx_stage.append(per_core)

    # --- build + run -------------------------------------------------------
    nc = _build(ns, caps, nblocks)
    in_maps = []
    for k in range(N_CORES):
        m = {}
        for g in range(len(GROUPS)):
            if ns[g] == 0:
                continue
            m[f"t{g}"] = emb_stage[g]
            m[f"i{g}"] = idx_stage[g][k]
        for pj in range(4):
            if any(ns[g] and GROUPS[g][6] == pj for g in range(len(GROUPS))):
                m[f"p{pj}"] = pjt_stage[pj]
        in_maps.append(m)

    res = run_bass_kernel_spmd(nc, in_maps, core_ids=list(range(N_CORES)))
    LAST_RESULT = res

    # --- unpermute ---------------------------------------------------------
    out_full = np.zeros((T, D_PROJ), np.float32)
    for k in range(N_CORES):
        rows = res.results[k]["out"]
        off = 0
        for g in range(len(GROUPS)):
            if ns[g] == 0:
                continue
            pos = pos_cores[g][k]
            valid = pos >= 0
            out_full[pos[valid]] = rows[off:off + ns[g]][valid]
            off += ns[g]
    return out_full.reshape(*inp.shape, D_PROJ)


# revision 2
# speedup vs baseline: 1.3636x; 1.3636x over previous
"""Adaptive embedding (nn_AdaptiveEmbedding) Trainium2 Bass kernel.

Strategy: token-shard across 8 NeuronCores. Host routes each token to one of
6 vocab groups (cluster 1 and 2 are split in half so local row indices fit
int16 for dma_gather), pads each group's token list so every core gets an
identical count, and stages:
  - bf16 tables (narrow tables padded to 128 cols so gather rows are >=256B)
  - bf16 pre-transposed projections (scaled by sqrt(D_PROJ)), chunk-major
  - per-core int16 gather index tensors (16-partition wrapped, replicated)
Device (per core): dma_gather(transpose=True) pulls embedding rows already
transposed [d, tokens]; TensorE contracts d against the projection chunks
into PSUM [tokens, 1024]; DVE copies to SBUF; one DMA per group writes the
output rows. Host scatters the per-core rows back to original positions.

Engine specialization: ACT=idx loads + output stores (HWDGE), SP=projection
loads (HWDGE), Pool=gathers (SWDGE), PE=matmul, DVE=psum->sbuf copies.
"""

import math

import numpy as np
import ml_dtypes

import concourse.bacc as bacc
import concourse.mybir as mybir
import concourse.tile as tile
from concourse.bass_utils import run_bass_kernel_spmd

N_CORES = 8
D_PROJ = 1024
EMB_SCALE = float(D_PROJ) ** 0.5

# (token_left, token_right, table_name, row_lo, row_hi, d, proj_index)
GROUPS = [
    (0, 20000, "emb0", 0, 20000, 1024, 0),
    (20000, 40000, "emb1", 0, 20000, 256, 1),
    (40000, 60000, "emb1", 20000, 40000, 256, 1),
    (60000, 80000, "emb2", 0, 20000, 64, 2),
    (80000, 100000, "emb2", 20000, 40000, 64, 2),
    (100000, 128000, "emb3", 0, 28000, 16, 3),
]
PJ_DIMS = [1024, 256, 64, 16]

BF16 = ml_dtypes.bfloat16

# Module-level handle for test harness inspection (exec_time_ns etc).
LAST_RESULT = None


def _elem_size(d):
    """Gathered row length in bf16 elements (>=256B granularity)."""
    return max(d, 128)


def _build(ns, caps, nblocks):
    """Build the SPMD Bass graph. ns[g]=valid tokens per core per group,
    caps[g]=gather count (ns rounded up to 128), nblocks[g]=caps/128."""
    nc = bacc.Bacc("TRN2", target_bir_lowering=False, num_devices=N_CORES)
    ng = len(GROUPS)
    act = [g for g in range(ng) if ns[g] > 0]

    embs, idxs = {}, {}
    for g in act:
        _, _, tname, rlo, rhi, d, pj = GROUPS[g]
        embs[g] = nc.dram_tensor(f"t{g}", [rhi - rlo, _elem_size(d)],
                                 mybir.dt.bfloat16, kind="ExternalInput")
        idxs[g] = nc.dram_tensor(f"i{g}", [128, caps[g] // 16], mybir.dt.int16,
                                 kind="ExternalInput")

    used_pj = sorted({GROUPS[g][6] for g in act})
    pjts = {}
    for pj in used_pj:
        d = PJ_DIMS[pj]
        pjts[pj] = nc.dram_tensor(
            f"p{pj}", [min(d, 128), max(d // 128, 1) * D_PROJ],
            mybir.dt.bfloat16, kind="ExternalInput")

    total_rows = sum(caps)
    out = nc.dram_tensor("out", [total_rows, D_PROJ], mybir.dt.float32,
                         kind="ExternalOutput")

    with tile.TileContext(nc) as tc:
        with (
            tc.tile_pool(name="const", bufs=1) as cpool,
            tc.tile_pool(name="psum", bufs=3, space="PSUM") as ppool,
        ):
            # 1) index loads (ACT HWDGE) — tiny, gate the gathers
            idx_t = {}
            for g in act:
                it = cpool.tile([128, caps[g] // 16], mybir.dt.int16,
                                tag=f"idx{g}")
                nc.scalar.dma_start(it[:], idxs[g][:])
                idx_t[g] = it

            # 2) gathers (Pool SWDGE), g0 first
            et_t = {}
            for g in act:
                d = GROUPS[g][5]
                es = _elem_size(d)
                nchunk = es // 128
                et = cpool.tile([128, nchunk, caps[g]], mybir.dt.bfloat16,
                                tag=f"et{g}")
                nc.gpsimd.dma_gather(et[:], embs[g][:], idx_t[g][:], caps[g],
                                     caps[g], es, transpose=True)
                et_t[g] = et

            # 3) projection loads (SP HWDGE), 2 chunks per DMA
            pjt_t = {}  # pj -> list of chunk tiles (one per 128-row k-chunk)
            for pj in used_pj:
                d = PJ_DIMS[pj]
                part = min(d, 128)
                nchunk = max(d // 128, 1)
                tiles = []
                for c0 in range(0, nchunk, 2):
                    w = min(2, nchunk - c0)
                    t = cpool.tile([part, w * D_PROJ], mybir.dt.bfloat16,
                                   tag=f"pjt{pj}_{c0}")
                    nc.sync.dma_start(
                        t[:], pjts[pj][:, c0 * D_PROJ:(c0 + w) * D_PROJ])
                    for i in range(w):
                        tiles.append((t, i))
                pjt_t[pj] = tiles

            # 4) matmul blocks + DVE copies + per-group output DMA
            off = 0
            for g in act:
                _, _, _, rlo, rhi, d, pj = GROUPS[g]
                kdim = min(d, 128)
                nchunk = max(d // 128, 1)
                og = cpool.tile([128, nblocks[g] * D_PROJ], mybir.dt.float32,
                                tag=f"og{g}")
                for b in range(nblocks[g]):
                    m = min(128, ns[g] - b * 128)
                    if m <= 0:
                        continue
                    ps = ppool.tile([128, D_PROJ], mybir.dt.float32, tag="ps")
                    for h in range(2):
                        for c in range(nchunk):
                            pt, ci = pjt_t[pj][c]
                            nc.tensor.matmul(
                                ps[:m, h * 512:(h + 1) * 512],
                                et_t[g][:kdim, c, b * 128: b * 128 + m],
                                pt[:kdim, ci * D_PROJ + h * 512:
                                   ci * D_PROJ + h * 512 + 512],
                                start=(c == 0),
                                stop=(c == nchunk - 1),
                            )
                    nc.vector.tensor_copy(
                        og[:m, b * D_PROJ:(b + 1) * D_PROJ], ps[:m, :])
                dram_ap = out[off:off + caps[g], :].rearrange(
                    "(b p) d -> p b d", p=128)
                nc.scalar.dma_start(dram_ap, og[:])
                off += caps[g]

    nc.compile()
    return nc


def kernel(input, emb0, emb1, emb2, emb3, proj0, proj1, proj2, proj3):
    global LAST_RESULT
    inp = np.asarray(input)
    flat = inp.reshape(-1).astype(np.int64)
    T = flat.shape[0]
    tables = {"emb0": np.asarray(emb0), "emb1": np.asarray(emb1),
              "emb2": np.asarray(emb2), "emb3": np.asarray(emb3)}
    projs = [np.asarray(proj0), np.asarray(proj1),
             np.asarray(proj2), np.asarray(proj3)]

    # --- route tokens to groups -------------------------------------------
    ns, caps, nblocks = [], [], []
    pos_cores, loc_cores = [], []  # per group: [8, n_g] arrays
    for (l, r, tname, rlo, rhi, d, pj) in GROUPS:
        sel = (flat >= l) & (flat < r)
        pos = np.nonzero(sel)[0]
        loc = (flat[pos] - l).astype(np.int64)
        n = int(math.ceil(len(pos) / N_CORES)) if len(pos) else 0
        ns.append(n)
        caps.append(((n + 127) // 128) * 128 if n else 0)
        nblocks.append((n + 127) // 128 if n else 0)
        pad = n * N_CORES - len(pos)
        pos_p = np.concatenate([pos, np.full(pad, -1, np.int64)])
        loc_p = np.concatenate([loc, np.zeros(pad, np.int64)])
        pos_cores.append(pos_p.reshape(N_CORES, n) if n else None)
        loc_cores.append(loc_p.reshape(N_CORES, n) if n else None)

    # --- stage tables (bf16, narrow rows padded to 128 cols) ---------------
    emb_stage = {}
    for g, (l, r, tname, rlo, rhi, d, pj) in enumerate(GROUPS):
        if ns[g] == 0:
            continue
        sl = tables[tname][rlo:rhi].astype(BF16)
        es = _elem_size(d)
        if es != d:
            padded = np.zeros((sl.shape[0], es), dtype=BF16)
            padded[:, :d] = sl
            sl = padded
        emb_stage[g] = np.ascontiguousarray(sl)

    # --- stage projections: (proj.T * EMB_SCALE), chunk-major bf16 ---------
    pjt_stage = [None] * 4
    for pj in range(4):
        d = projs[pj].shape[1]
        pt = (projs[pj].T.astype(np.float32) * EMB_SCALE)  # [d, D_PROJ]
        if d >= 128:
            nchunk = d // 128
            pt = pt.reshape(nchunk, 128, D_PROJ).transpose(1, 0, 2)
            pt = pt.reshape(128, nchunk * D_PROJ)
        pjt_stage[pj] = np.ascontiguousarray(pt.astype(BF16))

    # --- per-core index tensors -------------------------------------------
    idx_stage = {}  # [group][core] -> [128, cap/16] int16
    for g in range(len(GROUPS)):
        if ns[g] == 0:
            continue
        cap = caps[g]
        per_core = []
        for k in range(N_CORES):
            loc = loc_cores[g][k]
            full = np.zeros(cap, np.int16)
            full[:ns[g]] = loc.astype(np.int16)
            wrapped = np.tile(full.reshape(cap // 16, 16).T, (8, 1))
            per_core.append(np.ascontiguousarray(wrapped))
        idx_stage[g] = per_core

    # --- build + run -------------------------------------------------------
    nc = _build(ns, caps, nblocks)
    in_maps = []
    for k in range(N_CORES):
        m = {}
        for g in range(len(GROUPS)):
            if ns[g] == 0:
                continue
            m[f"t{g}"] = emb_stage[g]
            m[f"i{g}"] = idx_stage[g][k]
        for pj in sorted({GROUPS[g][6] for g in range(len(GROUPS)) if ns[g]}):
            m[f"p{pj}"] = pjt_stage[pj]
        in_maps.append(m)

    res = run_bass_kernel_spmd(nc, in_maps, core_ids=list(range(N_CORES)))
    LAST_RESULT = res

    # --- unpermute ---------------------------------------------------------
    out_full = np.zeros((T, D_PROJ), np.float32)
    for k in range(N_CORES):
        rows = res.results[k]["out"]
        off = 0
        for g in range(len(GROUPS)):
            if ns[g] == 0:
                continue
            pos = pos_cores[g][k]
            valid = pos >= 0
            out_full[pos[valid]] = rows[off:off + ns[g]][valid]
            off += caps[g]
    return out_full.reshape(*inp.shape, D_PROJ)


# revision 3
# speedup vs baseline: 1.4899x; 1.0926x over previous
"""Adaptive embedding (nn_AdaptiveEmbedding) Trainium2 Bass kernel.

Strategy: token-shard across 8 NeuronCores. Host routes each token to one of
6 vocab groups (cluster 1 and 2 are split in half so local row indices fit
int16 for dma_gather), pads each group's token list so every core gets an
identical count, and stages:
  - bf16 tables (narrow tables padded to 128 cols so gather rows are >=256B)
  - bf16 pre-transposed projections (scaled by sqrt(D_PROJ)), chunk-major
  - per-core int16 gather index tensors (16-partition wrapped, replicated);
    pad slots hold -1 so the gather ucode skips their descriptors
Device (per core): dma_gather(transpose=True) pulls embedding rows already
transposed [d, tokens]; TensorE contracts d against the projection chunks
into PSUM [tokens, 1024]; DVE copies (cast bf16) to SBUF; one DMA per group
writes output rows. Host scatters rows back to original token positions.

Engine specialization: ACT=idx loads + output stores (HWDGE), SP=projection
loads (HWDGE), Pool=gathers (SWDGE), PE=matmul, DVE=psum->sbuf copies.
"""

import math

import numpy as np
import ml_dtypes

import concourse.bacc as bacc
import concourse.mybir as mybir
import concourse.tile as tile
from concourse.bass_utils import run_bass_kernel_spmd
from concourse.library_config import mlp as _mlp_lib

N_CORES = 8
D_PROJ = 1024
EMB_SCALE = float(D_PROJ) ** 0.5

# (token_left, token_right, table_name, row_lo, row_hi, d, proj_index)
GROUPS = [
    (0, 20000, "emb0", 0, 20000, 1024, 0),
    (20000, 40000, "emb1", 0, 20000, 256, 1),
    (40000, 60000, "emb1", 20000, 40000, 256, 1),
    (60000, 80000, "emb2", 0, 20000, 64, 2),
    (80000, 100000, "emb2", 20000, 40000, 64, 2),
    (100000, 128000, "emb3", 0, 28000, 16, 3),
]
PJ_DIMS = [1024, 256, 64, 16]

BF16 = ml_dtypes.bfloat16

# Module-level handle for test harness inspection (exec_time_ns etc).
LAST_RESULT = None


def _elem_size(d):
    """Gathered row length in bf16 elements (>=256B granularity)."""
    return max(d, 128)


def _build(ns, caps, nblocks):
    """Build the SPMD Bass graph. ns[g]=valid tokens per core per group,
    caps[g]=gather count (ns rounded up to 128), nblocks[g]=caps/128."""
    nc = bacc.Bacc("TRN2", target_bir_lowering=False, num_devices=N_CORES)
    ng = len(GROUPS)
    act = [g for g in range(ng) if ns[g] > 0]

    embs = {}
    for g in act:
        _, _, tname, rlo, rhi, d, pj = GROUPS[g]
        embs[g] = nc.dram_tensor(f"t{g}", [rhi - rlo, _elem_size(d)],
                                 mybir.dt.bfloat16, kind="ExternalInput")

    idx_cols = sum(caps[g] // 16 for g in act)
    idx_all = nc.dram_tensor("idx", [128, idx_cols], mybir.dt.int16,
                             kind="ExternalInput")

    used_pj = sorted({GROUPS[g][6] for g in act})
    pjts = {}
    for pj in used_pj:
        d = PJ_DIMS[pj]
        pjts[pj] = nc.dram_tensor(
            f"p{pj}", [min(d, 128), max(d // 128, 1) * D_PROJ],
            mybir.dt.bfloat16, kind="ExternalInput")

    total_rows = sum(caps)
    out = nc.dram_tensor("out", [total_rows, D_PROJ], mybir.dt.bfloat16,
                         kind="ExternalOutput")

    with tile.TileContext(nc) as tc:
        with (
            tc.tile_pool(name="const", bufs=1) as cpool,
            tc.tile_pool(name="psum", bufs=3, space="PSUM") as ppool,
        ):
            # 0) start the gather ucode library load ASAP (it takes ~5us)
            nc.gpsimd.load_library(_mlp_lib)

            # 1) one combined index load (ACT HWDGE) — gates the gathers
            it_all = cpool.tile([128, idx_cols], mybir.dt.int16, tag="idx")
            nc.scalar.dma_start(it_all[:], idx_all[:])
            idx_off = {}
            o = 0
            for g in act:
                idx_off[g] = o
                o += caps[g] // 16

            # 2) gathers (Pool SWDGE), g0 first; pads are -1 => skipped
            et_t = {}
            for g in act:
                d = GROUPS[g][5]
                es = _elem_size(d)
                nchunk = es // 128
                et = cpool.tile([128, nchunk, caps[g]], mybir.dt.bfloat16,
                                tag=f"et{g}")
                nc.gpsimd.dma_gather(
                    et[:], embs[g][:],
                    it_all[:, idx_off[g]: idx_off[g] + caps[g] // 16],
                    caps[g], ns[g], es, transpose=True)
                et_t[g] = et

            # 3) projection loads (SP HWDGE), 2 chunks per DMA
            pjt_t = {}  # pj -> list of (tile, col-chunk) per 128-row k-chunk
            for pj in used_pj:
                d = PJ_DIMS[pj]
                part = min(d, 128)
                nchunk = max(d // 128, 1)
                tiles = []
                for c0 in range(0, nchunk, 2):
                    w = min(2, nchunk - c0)
                    t = cpool.tile([part, w * D_PROJ], mybir.dt.bfloat16,
                                   tag=f"pjt{pj}_{c0}")
                    nc.sync.dma_start(
                        t[:], pjts[pj][:, c0 * D_PROJ:(c0 + w) * D_PROJ])
                    for i in range(w):
                        tiles.append((t, i))
                pjt_t[pj] = tiles

            # 4) matmul blocks + DVE copies (cast bf16) + per-group out DMA
            off = 0
            for g in act:
                _, _, _, rlo, rhi, d, pj = GROUPS[g]
                kdim = min(d, 128)
                nchunk = max(d // 128, 1)
                og = cpool.tile([128, nblocks[g] * D_PROJ], mybir.dt.bfloat16,
                                tag=f"og{g}")
                for b in range(nblocks[g]):
                    m = min(128, ns[g] - b * 128)
                    if m <= 0:
                        continue
                    ps = ppool.tile([128, D_PROJ], mybir.dt.float32, tag="ps")
                    for h in range(2):
                        for c in range(nchunk):
                            pt, ci = pjt_t[pj][c]
                            nc.tensor.matmul(
                                ps[:m, h * 512:(h + 1) * 512],
                                et_t[g][:kdim, c, b * 128: b * 128 + m],
                                pt[:kdim, ci * D_PROJ + h * 512:
                                   ci * D_PROJ + h * 512 + 512],
                                start=(c == 0),
                                stop=(c == nchunk - 1),
                            )
                    nc.vector.tensor_copy(
                        og[:m, b * D_PROJ:(b + 1) * D_PROJ], ps[:m, :])
                dram_ap = out[off:off + caps[g], :].rearrange(
                    "(b p) d -> p b d", p=128)
                nc.scalar.dma_start(dram_ap, og[:])
                off += caps[g]

    nc.compile()
    return nc


def kernel(input, emb0, emb1, emb2, emb3, proj0, proj1, proj2, proj3):
    global LAST_RESULT
    inp = np.asarray(input)
    flat = inp.reshape(-1).astype(np.int64)
    T = flat.shape[0]
    tables = {"emb0": np.asarray(emb0), "emb1": np.asarray(emb1),
              "emb2": np.asarray(emb2), "emb3": np.asarray(emb3)}
    projs = [np.asarray(proj0), np.asarray(proj1),
             np.asarray(proj2), np.asarray(proj3)]

    # --- route tokens to groups -------------------------------------------
    ns, caps, nblocks = [], [], []
    pos_cores, loc_cores = [], []  # per group: [8, n_g] arrays
    for (l, r, tname, rlo, rhi, d, pj) in GROUPS:
        sel = (flat >= l) & (flat < r)
        pos = np.nonzero(sel)[0]
        loc = (flat[pos] - l).astype(np.int64)
        n = int(math.ceil(len(pos) / N_CORES)) if len(pos) else 0
        ns.append(n)
        caps.append(((n + 127) // 128) * 128 if n else 0)
        nblocks.append((n + 127) // 128 if n else 0)
        pad = n * N_CORES - len(pos)
        pos_p = np.concatenate([pos, np.full(pad, -1, np.int64)])
        loc_p = np.concatenate([loc, np.zeros(pad, np.int64)])
        pos_cores.append(pos_p.reshape(N_CORES, n) if n else None)
        loc_cores.append(loc_p.reshape(N_CORES, n) if n else None)

    act = [g for g in range(len(GROUPS)) if ns[g] > 0]

    # --- stage tables (bf16, narrow rows padded to 128 cols) ---------------
    emb_stage = {}
    for g in act:
        l, r, tname, rlo, rhi, d, pj = GROUPS[g]
        sl = tables[tname][rlo:rhi].astype(BF16)
        es = _elem_size(d)
        if es != d:
            padded = np.zeros((sl.shape[0], es), dtype=BF16)
            padded[:, :d] = sl
            sl = padded
        emb_stage[g] = np.ascontiguousarray(sl)

    # --- stage projections: (proj.T * EMB_SCALE), chunk-major bf16 ---------
    pjt_stage = [None] * 4
    for pj in range(4):
        d = projs[pj].shape[1]
        pt = (projs[pj].T.astype(np.float32) * EMB_SCALE)  # [d, D_PROJ]
        if d >= 128:
            nchunk = d // 128
            pt = pt.reshape(nchunk, 128, D_PROJ).transpose(1, 0, 2)
            pt = pt.reshape(128, nchunk * D_PROJ)
        pjt_stage[pj] = np.ascontiguousarray(pt.astype(BF16))

    # --- per-core combined index tensor (pads = -1) ------------------------
    idx_stage = []  # per core -> [128, sum(caps)/16] int16
    for k in range(N_CORES):
        parts = []
        for g in act:
            cap = caps[g]
            full = np.full(cap, -1, np.int16)
            full[:ns[g]] = loc_cores[g][k].astype(np.int16)
            parts.append(np.tile(full.reshape(cap // 16, 16).T, (8, 1)))
        idx_stage.append(np.ascontiguousarray(np.concatenate(parts, axis=1)))

    # --- build + run -------------------------------------------------------
    nc = _build(ns, caps, nblocks)
    in_maps = []
    for k in range(N_CORES):
        m = {"idx": idx_stage[k]}
        for g in act:
            m[f"t{g}"] = emb_stage[g]
        for pj in sorted({GROUPS[g][6] for g in act}):
            m[f"p{pj}"] = pjt_stage[pj]
        in_maps.append(m)

    res = run_bass_kernel_spmd(nc, in_maps, core_ids=list(range(N_CORES)))
    LAST_RESULT = res

    # --- unpermute ---------------------------------------------------------
    out_full = np.zeros((T, D_PROJ), np.float32)
    for k in range(N_CORES):
        rows = res.results[k]["out"]
        off = 0
        for g in act:
            pos = pos_cores[g][k]
            valid = pos >= 0
            out_full[pos[valid]] = rows[off:off + ns[g]][valid].astype(
                np.float32)
            off += caps[g]
    return out_full.reshape(*inp.shape, D_PROJ)


# revision 4
# speedup vs baseline: 1.5779x; 1.0591x over previous
"""Adaptive embedding (nn_AdaptiveEmbedding) Trainium2 Bass kernel.

Strategy: token-shard across 8 NeuronCores. Host routes each token to one of
6 vocab groups (cluster 1 and 2 are split in half so local row indices fit
int16 for dma_gather), pads each group's token list so every core gets an
identical count, and stages:
  - bf16 tables (narrow tables padded to 128 cols so gather rows are >=256B)
  - bf16 pre-transposed projections (scaled by sqrt(D_PROJ)), chunk-major
  - per-core int16 gather index tensors (16-partition wrapped, replicated);
    pad slots hold -1 so the gather ucode skips their descriptors
Device (per core), raw bass with explicit semaphores (no Tile overhead):
  Pool   : load gather ucode library, then 6 dma_gathers (transposed)
  SP     : projection chunk loads
  ACT    : index load first, then per-group output stores
  PE     : per 128-token block, contract d against projection chunks in PSUM
  DVE    : PSUM -> SBUF copies (cast bf16)
Host scatters the per-core output rows back to original token positions.
"""

import math
from contextlib import ExitStack

import numpy as np
import ml_dtypes

import concourse.bacc as bacc
import concourse.mybir as mybir
from concourse.bass_utils import run_bass_kernel_spmd
from concourse.library_config import mlp as _mlp_lib

N_CORES = 8
D_PROJ = 1024
EMB_SCALE = float(D_PROJ) ** 0.5

# (token_left, token_right, table_name, row_lo, row_hi, d, proj_index)
GROUPS = [
    (0, 20000, "emb0", 0, 20000, 1024, 0),
    (20000, 40000, "emb1", 0, 20000, 256, 1),
    (40000, 60000, "emb1", 20000, 40000, 256, 1),
    (60000, 80000, "emb2", 0, 20000, 64, 2),
    (80000, 100000, "emb2", 20000, 40000, 64, 2),
    (100000, 128000, "emb3", 0, 28000, 16, 3),
]
PJ_DIMS = [1024, 256, 64, 16]

BF16 = ml_dtypes.bfloat16
NPS = 4  # rotating PSUM tiles ([128,1024] f32 = 2 banks each)

# Module-level handle for test harness inspection (exec_time_ns etc).
LAST_RESULT = None


def _elem_size(d):
    """Gathered row length in bf16 elements (>=256B granularity)."""
    return max(d, 128)


def _build(ns, caps, nblocks):
    """Raw-bass SPMD graph. ns[g]=valid tokens per core per group,
    caps[g]=gather slot count (ns rounded to 128), nblocks[g]=caps/128."""
    nc = bacc.Bacc("TRN2", target_bir_lowering=False, num_devices=N_CORES)
    act = [g for g in range(len(GROUPS)) if ns[g] > 0]

    embs = {g: nc.dram_tensor(f"t{g}",
                              [GROUPS[g][4] - GROUPS[g][3],
                               _elem_size(GROUPS[g][5])],
                              mybir.dt.bfloat16, kind="ExternalInput")
            for g in act}
    idx_cols = sum(caps[g] // 16 for g in act)
    idx_all = nc.dram_tensor("idx", [128, idx_cols], mybir.dt.int16,
                             kind="ExternalInput")
    used_pj = sorted({GROUPS[g][6] for g in act})
    pjts = {pj: nc.dram_tensor(f"p{pj}",
                               [min(PJ_DIMS[pj], 128),
                                max(PJ_DIMS[pj] // 128, 1) * D_PROJ],
                               mybir.dt.bfloat16, kind="ExternalInput")
            for pj in used_pj}
    out = nc.dram_tensor("out", [sum(caps), D_PROJ], mybir.dt.bfloat16,
                         kind="ExternalOutput")

    stack = ExitStack()
    sb = lambda name, shape, dt: stack.enter_context(
        nc.sbuf_tensor(name, list(shape), dt))
    pt_ = lambda name, shape, dt: stack.enter_context(
        nc.psum_tensor(name, list(shape), dt))
    sem = lambda name: stack.enter_context(nc.semaphore(name))

    with stack:
        it_all = sb("idxs", [128, idx_cols], mybir.dt.int16)
        et_t, idx_off = {}, {}
        o = 0
        for g in act:
            es = _elem_size(GROUPS[g][5])
            et_t[g] = sb(f"et{g}", [128, es // 128, caps[g]],
                         mybir.dt.bfloat16)
            idx_off[g] = o
            o += caps[g] // 16
        # projection chunk tiles: pj -> list of (tile, col-chunk, dma_idx)
        pjt_t = {}
        n_pj_dma = 0
        for pj in used_pj:
            d = PJ_DIMS[pj]
            part, nchunk = min(d, 128), max(d // 128, 1)
            tiles = []
            for c0 in range(0, nchunk, 2):
                w = min(2, nchunk - c0)
                t = sb(f"pjt{pj}_{c0}", [part, w * D_PROJ], mybir.dt.bfloat16)
                n_pj_dma += 1
                for i in range(w):
                    tiles.append((t, i, n_pj_dma))
            pjt_t[pj] = tiles
        og_t = {g: sb(f"og{g}", [128, nblocks[g] * D_PROJ],
                      mybir.dt.bfloat16) for g in act}
        ps_t = [pt_(f"ps{i}", [128, D_PROJ], mybir.dt.float32)
                for i in range(NPS)]

        isem = sem("isem")
        psem = sem("psem")
        gsem = {g: sem(f"gsem{g}") for g in act}
        mm_sem = sem("mm_sem")
        cp_sem = sem("cp_sem")
        osem = sem("osem")

        # block schedule shared by PE/DVE/ACT
        blocks = []  # (g, b, m)
        for g in act:
            for b in range(nblocks[g]):
                m = min(128, ns[g] - b * 128)
                if m > 0:
                    blocks.append((g, b, m))
        cum_blocks = {}
        c = 0
        for g in act:
            c += sum(1 for gg, _, _ in blocks if gg == g)
            cum_blocks[g] = c

        with nc.Block(name="main") as block:

            @block.gpsimd
            def _(gp):
                gp.load_library(_mlp_lib)
                gp.wait_ge(isem, 16)
                for g in act:
                    es = _elem_size(GROUPS[g][5])
                    gp.dma_gather(
                        et_t[g][:], embs[g][:],
                        it_all[:, idx_off[g]: idx_off[g] + caps[g] // 16],
                        caps[g], ns[g], es, transpose=True,
                    ).then_inc(gsem[g], 16)

            @block.scalar
            def _(sc):
                sc.dma_start(it_all[:], idx_all[:]).then_inc(isem, 16)
                off = 0
                for g in act:
                    sc.wait_ge(cp_sem, cum_blocks[g])
                    dram_ap = out[off:off + caps[g], :].rearrange(
                        "(b p) d -> p b d", p=128)
                    sc.dma_start(dram_ap, og_t[g][:]).then_inc(osem, 16)
                    off += caps[g]
                sc.wait_ge(osem, 16 * len(act))

            @block.sync
            def _(sy):
                for pj in used_pj:
                    d = PJ_DIMS[pj]
                    nchunk = max(d // 128, 1)
                    for c0 in range(0, nchunk, 2):
                        w = min(2, nchunk - c0)
                        t = pjt_t[pj][c0][0]
                        sy.dma_start(
                            t[:], pjts[pj][:, c0 * D_PROJ:(c0 + w) * D_PROJ]
                        ).then_inc(psem, 16)

            @block.tensor
            def _(te):
                seen_g = set()
                for i, (g, b, m) in enumerate(blocks):
                    d = GROUPS[g][5]
                    pj = GROUPS[g][6]
                    kdim, nchunk = min(d, 128), max(d // 128, 1)
                    if g not in seen_g:
                        seen_g.add(g)
                        te.wait_ge(gsem[g], 16)
                        need = max(dma_i for _, _, dma_i in pjt_t[pj])
                        te.wait_ge(psem, 16 * need)
                    if i >= NPS:
                        te.wait_ge(cp_sem, i - NPS + 1)
                    ps = ps_t[i % NPS]
                    last = None
                    for h in range(2):
                        for c in range(nchunk):
                            pt, ci, _ = pjt_t[pj][c]
                            last = te.matmul(
                                ps[:m, h * 512:(h + 1) * 512],
                                et_t[g][:kdim, c, b * 128: b * 128 + m],
                                pt[:kdim, ci * D_PROJ + h * 512:
                                   ci * D_PROJ + h * 512 + 512],
                                start=(c == 0),
                                stop=(c == nchunk - 1),
                            )
                    last.then_inc(mm_sem, 1)

            @block.vector
            def _(ve):
                for i, (g, b, m) in enumerate(blocks):
                    ve.wait_ge(mm_sem, i + 1)
                    ve.tensor_copy(
                        og_t[g][:m, b * D_PROJ:(b + 1) * D_PROJ],
                        ps_t[i % NPS][:m, :],
                    ).then_inc(cp_sem, 1)

        nc.compile()
    return nc


def kernel(input, emb0, emb1, emb2, emb3, proj0, proj1, proj2, proj3):
    global LAST_RESULT
    inp = np.asarray(input)
    flat = inp.reshape(-1).astype(np.int64)
    T = flat.shape[0]
    tables = {"emb0": np.asarray(emb0), "emb1": np.asarray(emb1),
              "emb2": np.asarray(emb2), "emb3": np.asarray(emb3)}
    projs = [np.asarray(proj0), np.asarray(proj1),
             np.asarray(proj2), np.asarray(proj3)]

    # --- route tokens to groups -------------------------------------------
    ns, caps, nblocks = [], [], []
    pos_cores, loc_cores = [], []  # per group: [8, n_g] arrays
    for (l, r, tname, rlo, rhi, d, pj) in GROUPS:
        sel = (flat >= l) & (flat < r)
        pos = np.nonzero(sel)[0]
        loc = (flat[pos] - l).astype(np.int64)
        n = int(math.ceil(len(pos) / N_CORES)) if len(pos) else 0
        ns.append(n)
        caps.append(((n + 127) // 128) * 128 if n else 0)
        nblocks.append((n + 127) // 128 if n else 0)
        pad = n * N_CORES - len(pos)
        pos_p = np.concatenate([pos, np.full(pad, -1, np.int64)])
        loc_p = np.concatenate([loc, np.zeros(pad, np.int64)])
        pos_cores.append(pos_p.reshape(N_CORES, n) if n else None)
        loc_cores.append(loc_p.reshape(N_CORES, n) if n else None)

    act = [g for g in range(len(GROUPS)) if ns[g] > 0]

    # --- stage tables (bf16, narrow rows padded to 128 cols) ---------------
    emb_stage = {}
    for g in act:
        l, r, tname, rlo, rhi, d, pj = GROUPS[g]
        sl = tables[tname][rlo:rhi].astype(BF16)
        es = _elem_size(d)
        if es != d:
            padded = np.zeros((sl.shape[0], es), dtype=BF16)
            padded[:, :d] = sl
            sl = padded
        emb_stage[g] = np.ascontiguousarray(sl)

    # --- stage projections: (proj.T * EMB_SCALE), chunk-major bf16 ---------
    pjt_stage = [None] * 4
    for pj in range(4):
        d = projs[pj].shape[1]
        pt = (projs[pj].T.astype(np.float32) * EMB_SCALE)  # [d, D_PROJ]
        if d >= 128:
            nchunk = d // 128
            pt = pt.reshape(nchunk, 128, D_PROJ).transpose(1, 0, 2)
            pt = pt.reshape(128, nchunk * D_PROJ)
        pjt_stage[pj] = np.ascontiguousarray(pt.astype(BF16))

    # --- per-core combined index tensor (pads = -1) ------------------------
    idx_stage = []  # per core -> [128, sum(caps)/16] int16
    for k in range(N_CORES):
        parts = []
        for g in act:
            cap = caps[g]
            full = np.full(cap, -1, np.int16)
            full[:ns[g]] = loc_cores[g][k].astype(np.int16)
            parts.append(np.tile(full.reshape(cap // 16, 16).T, (8, 1)))
        idx_stage.append(np.ascontiguousarray(np.concatenate(parts, axis=1)))

    # --- build + run -------------------------------------------------------
    nc = _build(ns, caps, nblocks)
    in_maps = []
    for k in range(N_CORES):
        m = {"idx": idx_stage[k]}
        for g in act:
            m[f"t{g}"] = emb_stage[g]
        for pj in sorted({GROUPS[g][6] for g in act}):
            m[f"p{pj}"] = pjt_stage[pj]
        in_maps.append(m)

    res = run_bass_kernel_spmd(nc, in_maps, core_ids=list(range(N_CORES)))
    LAST_RESULT = res

    # --- unpermute ---------------------------------------------------------
    out_full = np.zeros((T, D_PROJ), np.float32)
    for k in range(N_CORES):
        rows = res.results[k]["out"]
        off = 0
        for g in act:
            pos = pos_cores[g][k]
            valid = pos >= 0
            out_full[pos[valid]] = rows[off:off + ns[g]][valid].astype(
                np.float32)
            off += caps[g]
    return out_full.reshape(*inp.shape, D_PROJ)
